# revision 22
# baseline (speedup 1.0000x reference)
"""Multi-head causal self-attention on 8 Trainium2 NeuronCores.

Problem: x[4,2048,1024] @ w_qkv[1024,3072] -> 16-head causal attention
         -> @ w_out[1024,1024] + b_out.

Sharding (hardcoded): 8 cores = 4 batches x 2 head-groups of 8 heads.
Core c handles batch b = c//2 and heads hg*8..hg*8+8, hg = c%2.
Each core computes a partial output [2048,1024] (its 8 heads pushed
through its w_out row-slice); host sums the two head-group f16 partials
per batch and adds b_out.

Projections run in fp16; the attention pipeline (qT/kT/v/p/att/wo) is
bf16 so DVE tensor_tensor ops qualify for the 2x uop -- which only
triggers for non-in-place ops, hence the p-sum accumulation ping-pongs
between fresh buffers instead of updating in place. ~4e-3 relative
error vs the fp32 reference. (fp8 e4m3 QK via DoubleRow was evaluated
and rejected: ~3.3e-2 end-to-end error.)

Machine model (measured): every matmul streams 1 rhs row/cycle at
~2.4GHz; 64-deep pairs at tile positions (0,0)/(64,0) or (0,0)/(0,64)
stream BOTH rhs concurrently (2 rows/cycle aggregate), so QK/PV pairs
and the full-128-deep projection matmuls all run at full array rate.
The wall clock is essentially the PE queue makespan: streaming floor
~173us plus pipeline-drain exposure wherever a matmul waits on the
exp chain, plus startup/endgame. Hence the scheduling-oriented design:

  - Host pre-permutes every weight/x chunk so each input tensor is a
    partition-major [128, ...] DRAM block. Latency-critical tensors
    (wq, x^T chunk 0, wk, wv) load as piece-PAIR DMAs alternating
    across both hardware queues -- two 128-row pieces per ~565ns issue
    slot doubles the arrival rate while keeping fine-grained
    completion semaphores, so the first projection groups stream
    behind the DMAs (one bulk transfer per tensor stalled them ~5us
    on its single end-of-transfer semaphore). Far-from-critical
    tensors (x^T chunks 1-3, w_out, mask) load as one large DMA each,
    cutting ~35 DMA issue slots off the two queues.
  - Eight dummy matmuls on a memset scratch tile run while the first
    DMA pieces are in flight, plus four more in the endgame reciprocal
    stall, so the PE enters (and re-enters) its fast p-state before
    real work arrives (cold matmuls are ~1.6x slower).
  - Within each attention j-step, BOTH pairs' QK matmuls emit before
    either pair's PV: the in-order PE queue then never holds a PV
    (blocked on its exp) ahead of the other pair's QK, and the two
    exps pipeline back-to-back on ScalarE.
  - DVE (the second-busiest engine) op count is cut by fusing each
    diagonal tile's two per-head mask multiplies / p-sum adds into
    single strided two-region APs (~180ns fixed cost per op saved;
    head 1's exp output is PACKED at column 512, so its source regions
    use stride 512-ioff against the accumulator's natural stride 512).
  - opool bufs=8 so out-projection PE groups never wait on output-DMA
    ring slots.

Device algorithm per core (all "transposed orientation" so the only
transpose needed -- x^T -- is done for free on the host):
  qT/kT [512, 2048] and v (natural [2048, 512]) via fp16 matmuls.
  Attention runs per 512-query chunk with the two head-PAIRS of a duo
  interleaved j-tile by j-tile. Per pair, per key tile:
    scores^T for both heads go into one 2-bank PSUM tile with head 1's
    valid region packed adjacent to head 0's (no dead zone), both QK
    matmuls causally narrowed, streaming concurrently at PE tile
    positions (0,0)/(64,0). ONE exp per key tile on ScalarE covers
    both heads; 0/1-mask multiplies (DVE) only on diagonal-band tiles;
    out^T[d,i] += col-tiled PV matmuls at positions (0,0)/(0,64)
    (concurrent); p-sums accumulate in SBUF (DVE adds).
  Denominators: one M=1 ones-matmul per head partition-reduces the
  p-sum into a retired PV-output-ring PSUM slot; both pairs' den
  matmuls and unnormalized att copies emit before either pair's
  normalization so nothing queues behind the first pair's reciprocal
  chain. (GpSimd partition_all_reduce was measured at 8us per
  [128,1024] reduce -- far too slow to replace the ones-matmuls.)
  Per-pair normalization: reciprocal_approx_fast (DVE custom op), then
  a partition broadcast of the two reciprocal rows:
    - chunks 0-2 (hidden by later compute): strided 2-row DMA to a
      DRAM bounce, broadcast-read back to 128 partitions, in-place
      multiply into att^T.
    - chunk 3's last duo (nothing left to hide it): cast the two
      reciprocal rows to bf16 and splat them across partitions with
      two P=1 bf16 ones-matmuls into a retired PSUM slot (quadrants
      (0,0)/(64,64); head 1's denominator lands at partition 64 for
      this duo so the rows line up), and the in-place multiply reads
      PSUM directly. This cuts the final norm->out-projection critical
      path from ~7us (DMA round trip through DRAM) to ~2us; the
      unnormalized att copies for this duo ride ScalarE (idle after
      the last exp) so DVE reaches the reciprocals sooner. bf16
      reciprocals (vs f32) only touch 1/8th of the output at ~0.2%
      scale error -- noise against the 4e-3 overall error.
  partial = att^T.T @ w_out_slice -> f16 -> DMA to DRAM (the final
  projection call evacuates PSUM via ScalarE and splits its output
  DMAs across both queues, as ScalarE is idle after the last chunk).

PSUM budget (8 banks): score ring 2x[128,1024] = 4, shared
projection/QKV ring 2x[128,512] = 2, PV-output ring 2x[128,512] = 2.

Emission order A0h1 B0d0 A0h2 B0d1 A1 B1 C0 A2 B2 C1 A3 B3 C2 C3
keeps PE filler work (QKV projections / out-projections) available
inside every exp-paced attention chunk -- B(3), the longest, gets
C(2)+C(3) -- and splits A0/B0 so B0's first duo (which only needs q/k
pairs 0-1 and v) overlaps A0's remaining projection groups.
(Deferring ALL C phases into B(3) was tried and regressed 35us: the
extra evacuation casts head-of-line block B3's mask multiplies on the
in-order DVE queue and stall the QK score ring.)

Measured (this session's fast thermal regime): ~268-270us (very
stable at ~269.5) vs the ~283-285us session baseline; the part
intermittently throttles to a ~320us regime regardless of kernel
structure.
"""

import os
import sys

import ml_dtypes
import numpy as np

if "/opt/trn_rl_repo" not in sys.path:
    sys.path.insert(0, "/opt/trn_rl_repo")

B, T, C = 4, 2048, 1024
H, D = 16, 64
NCORES = 8
HPC = 8  # heads per core
PAIRS = 4  # head pairs per core
CCH = 8  # contraction chunks over C (1024/128)
ICH = 4  # i (query) chunks of 512
NJT = 16  # j (key) tiles of 128

_CACHE = {}


def _build_program():
    import concourse.mybir as mybir
    import concourse.tile as tile
    from concourse import bacc
    from concourse.bass import AP

    f32 = mybir.dt.float32
    bf16 = mybir.dt.bfloat16
    f16 = mybir.dt.float16
    EXP = mybir.ActivationFunctionType.Exp

    nc = bacc.Bacc(
        "TRN2", target_bir_lowering=False, debug=False, num_devices=NCORES
    )
    # Host pre-permutes so each DRAM tensor is partition-major: one
    # contiguous [128, ...] block per load (8KB descriptor lines).
    xt = nc.dram_tensor("xt", [ICH, 128, CCH, 512], f16, kind="ExternalInput").ap()
    wq = nc.dram_tensor("wq", [128, CCH, 512], f16, kind="ExternalInput").ap()
    wk = nc.dram_tensor("wk", [128, CCH, 512], f16, kind="ExternalInput").ap()
    wv = nc.dram_tensor("wv", [128, CCH, 512], f16, kind="ExternalInput").ap()
    wo = nc.dram_tensor("wo", [128, 4, C], bf16, kind="ExternalInput").ap()
    msk = nc.dram_tensor("msk", [128, 896], bf16, kind="ExternalInput").ap()
    out = nc.dram_tensor("out", [T, C], f16, kind="ExternalOutput").ap()

    with tile.TileContext(nc) as tc:
        with (
            tc.tile_pool(name="wpool", bufs=2) as wpool,
            tc.tile_pool(name="wvpool", bufs=1) as wvpool,
            tc.tile_pool(name="wopool", bufs=1) as wopool,
            tc.tile_pool(name="xpool", bufs=1) as xpool,
            tc.tile_pool(name="qkpool", bufs=8) as qkpool,
            tc.tile_pool(name="vpool", bufs=16) as vpool,
            tc.tile_pool(name="apool", bufs=4) as apool,
            tc.tile_pool(name="ppool", bufs=12) as ppool,
            tc.tile_pool(name="cpool", bufs=1) as cpool,
            tc.tile_pool(name="rpool", bufs=4) as rpool,
            tc.tile_pool(name="qpool", bufs=4) as qpool,
            tc.tile_pool(name="opool", bufs=8) as opool,
            tc.tile_pool(name="dpool", bufs=4, space="DRAM") as dpool,
            tc.tile_pool(name="ps_a", bufs=2, space="PSUM") as ps_a,
            tc.tile_pool(name="ps_s", bufs=2, space="PSUM") as ps_s,
            tc.tile_pool(name="ps_o", bufs=2, space="PSUM") as ps_o,
        ):
            # ---- constants ----
            ones_sb = cpool.tile([128, 1], bf16, name="ones_sb")
            nc.vector.memset(ones_sb, 1.0)
            # bf16 ones for the endgame reciprocal splat matmuls (bf16
            # keeps the splat single-pass; fp32 matmuls lower to a 2x
            # LOW/HIGH decomposition).
            ones_bf = cpool.tile([128, 64], bf16, name="ones_bf")
            nc.vector.memset(ones_bf, 1.0)
            # Scratch operand for PE warm-up matmuls (result discarded).
            scratch = cpool.tile([128, 512], bf16, name="scratch")
            nc.gpsimd.memset(scratch, 0.0)
            mask_sb = cpool.tile([128, 896], bf16, name="mask_sb")

            # PE warm-up: six dummy 512-row matmuls while the first input
            # pieces are still in flight, so the PE reaches its fast
            # p-state before the first real projection group.
            ps_w = ps_s.tile([128, 512], f32, name="ps_w", tag="pss")
            for i in range(12):
                nc.tensor.matmul(
                    ps_w,
                    lhsT=scratch[:, 0:128],
                    rhs=scratch[:],
                    start=(i == 0),
                    stop=(i == 11),
                )

            # Merged persistent input tiles.
            xt_all = xpool.tile([128, CCH, T], f16, name="xt_all", tag="xt")
            wq_all = wpool.tile([128, CCH, 512], f16, name="wq_all", tag="w")
            wk_all = wpool.tile([128, CCH, 512], f16, name="wk_all", tag="w")
            wv_all = wvpool.tile([128, CCH, 512], f16, name="wv_all", tag="wv")
            wo_all = wopool.tile([128, 4, C], bf16, name="wo_all", tag="wo")

            # wq + x^T chunk-0 pieces alternate across BOTH hardware
            # queues: matmul cc=k of the first group only needs piece k of
            # each, so the first A group streams behind the DMAs at the
            # two-queue rate.
            for cp in range(CCH // 2):
                ea, eb = (nc.sync, nc.scalar) if cp % 2 == 0 else (
                    nc.scalar,
                    nc.sync,
                )
                ea.dma_start(
                    out=wq_all[:, 2 * cp : 2 * cp + 2, :],
                    in_=wq[:, 2 * cp : 2 * cp + 2, :],
                )
                eb.dma_start(
                    out=xt_all[:, 2 * cp : 2 * cp + 2, 0:512],
                    in_=xt[0, :, 2 * cp : 2 * cp + 2, :],
                )
            # wk/wv stay piece-wise (alternating hardware queues) so the
            # k/v projection groups stream behind per-piece completion
            # semaphores instead of waiting on one bulk transfer. (GpSimd
            # SWDGE as a third issue queue was tried for wk and regressed
            # ~5us: ~1us/piece descriptor generation delivers too slowly.)
            for cp in range(CCH // 2):
                ea = nc.sync if cp % 2 == 0 else nc.scalar
                ea.dma_start(
                    out=wk_all[:, 2 * cp : 2 * cp + 2, :],
                    in_=wk[:, 2 * cp : 2 * cp + 2, :],
                )
            for cp in range(CCH // 2):
                ea = nc.scalar if cp % 2 == 0 else nc.sync
                ea.dma_start(
                    out=wv_all[:, 2 * cp : 2 * cp + 2, :],
                    in_=wv[:, 2 * cp : 2 * cp + 2, :],
                )
            nc.sync.dma_start(out=mask_sb, in_=msk)
            nc.scalar.dma_start(
                out=xt_all[:, :, 512:1024], in_=xt[1, :, :, :]
            )
            nc.sync.dma_start(
                out=xt_all[:, :, 1024:1536], in_=xt[2, :, :, :]
            )
            nc.scalar.dma_start(
                out=xt_all[:, :, 1536:2048], in_=xt[3, :, :, :]
            )
            nc.sync.dma_start(out=wo_all[:], in_=wo[:])

            w_sb = {}
            for cc in range(CCH):
                w_sb["wq", cc] = wq_all[:, cc, :]
                w_sb["wk", cc] = wk_all[:, cc, :]
                w_sb["wv", cc] = wv_all[:, cc, :]
            wo_sb = [wo_all[:, fc, :] for fc in range(4)]
            xt_sb = [xt_all[:, cc, :] for cc in range(CCH)]

            # ---- persistent activations ----
            qT = [
                qkpool.tile([128, T], bf16, name=f"qT_{p}", tag="qk")
                for p in range(PAIRS)
            ]
            kT = [
                qkpool.tile([128, T], bf16, name=f"kT_{p}", tag="qk")
                for p in range(PAIRS)
            ]
            v_sb = [
                vpool.tile([128, 512], bf16, name=f"v_{j}", tag="v")
                for j in range(NJT)
            ]
            att = [
                apool.tile([128, T], bf16, name=f"att_{p}", tag="att")
                for p in range(PAIRS)
            ]

            def phase_a(t4, pairs_sel=range(PAIRS), do_v=True):
                """QKV projections for token chunk t4 (512 tokens)."""
                tsl4 = slice(t4 * 512, (t4 + 1) * 512)
                xts = [xt_sb[cc][:, tsl4] for cc in range(CCH)]
                for wname, dst in (("wq", qT), ("wk", kT)):
                    for n in pairs_sel:
                        ps = ps_a.tile([128, 512], f32, name="ps_qk", tag="psA")
                        for cc in range(CCH):
                            nc.tensor.matmul(
                                ps,
                                lhsT=w_sb[wname, cc][:, n * 128 : (n + 1) * 128],
                                rhs=xts[cc][:],
                                start=(cc == 0),
                                stop=(cc == CCH - 1),
                            )
                        nc.vector.tensor_copy(
                            dst[n][:, t4 * 512 : (t4 + 1) * 512], ps
                        )
                if not do_v:
                    return
                for tt in range(4):
                    ps = ps_a.tile([128, 512], f32, name="ps_v", tag="psA")
                    for cc in range(CCH):
                        nc.tensor.matmul(
                            ps,
                            lhsT=xts[cc][:, tt * 128 : (tt + 1) * 128],
                            rhs=w_sb["wv", cc][:],
                            start=(cc == 0),
                            stop=(cc == CCH - 1),
                        )
                    nc.vector.tensor_copy(v_sb[t4 * 4 + tt], ps)

            def phase_b(ic, duo_sel=(0, 1)):
                """Attention for query chunk ic (512 queries).

                Pairs run as two interleaved duos (0,1) then (2,3): the
                j-tile loops of the duo alternate at emission so one pair's
                PV work hides the other pair's exp latency and the PE never
                idles long enough to drop out of its fast p-state.
                """
                isl = slice(ic * 512, (ic + 1) * 512)
                njt = 4 * ic + 4
                endgame = ic == 3
                duos = duo_sel

                def norm_pair(pr, den):
                    """1/denominators for pair pr -> broadcast + in-place mul."""
                    asl = att[pr][:, isl]
                    rec = rpool.tile([128, 1024], f32, name="rec", tag="rec")
                    if endgame and pr >= 2:
                        # Nothing left to hide a DRAM round trip behind:
                        # splat the two reciprocal rows (partitions 0/64)
                        # across partitions with two P=1 bf16 matmuls into
                        # a retired PSUM slot and multiply straight from
                        # PSUM.
                        nc.vector.reciprocal_approx_fast(
                            rec[0:65, 0:512], den[0:65, 0:512]
                        )
                        rec_bf = rpool.tile(
                            [128, 512], bf16, name="rec_bf", tag="recb"
                        )
                        nc.vector.tensor_copy(
                            rec_bf[0:1, :], rec[0:1, 0:512]
                        )
                        nc.vector.tensor_copy(
                            rec_bf[64:65, :], rec[64:65, 0:512]
                        )
                        rdb = ps_o.tile([128, 512], f32, name="rdb", tag="pso")
                        nc.tensor.matmul(
                            rdb[0:64, :],
                            lhsT=ones_bf[0:1, :],
                            rhs=rec_bf[0:1, :],
                            start=True,
                            stop=True,
                            tile_position=(0, 0),
                            skip_group_check=True,
                        )
                        nc.tensor.matmul(
                            rdb[64:128, :],
                            lhsT=ones_bf[64:65, :],
                            rhs=rec_bf[64:65, :],
                            start=True,
                            stop=True,
                            tile_position=(64, 64),
                            skip_group_check=True,
                        )
                        nc.vector.tensor_mul(asl, asl, rdb[:, 0:512])
                        return
                    eng = nc.sync
                    nc.vector.reciprocal_approx_fast(
                        rec[0:33, 0:512], den[0:33, 0:512]
                    )
                    dsc = dpool.tile([2, 512], f32, name="dsc", tag="dsc")
                    eng.dma_start(out=dsc, in_=rec[0:33:32, 0:512])
                    rdb = rpool.tile([128, 512], f32, name="rdb", tag="rdb")
                    eng.dma_start(
                        out=rdb[0:64, :],
                        in_=dsc[0:1, :].broadcast_to([64, 512]),
                    )
                    eng.dma_start(
                        out=rdb[64:128, :],
                        in_=dsc[1:2, :].broadcast_to([64, 512]),
                    )
                    nc.vector.tensor_mul(asl, asl, rdb)

                def two_region(t, off, rstride, width):
                    """[128, 2, width] view of t's two head spans at
                    offsets off and off+rstride."""
                    base = t[:, 0:1024]
                    return AP(
                        base.tensor,
                        base.offset + off,
                        [list(base.ap[0]), [rstride, 2], [1, width]],
                    )

                def mask_bcast(off, width):
                    base = mask_sb[:, off : off + width]
                    return AP(
                        base.tensor,
                        base.offset,
                        [list(base.ap[0]), [0, 2], [1, width]],
                    )

                def qk_exp(pr, jt, sb, pTb):
                    jsl = slice(jt * 128, (jt + 1) * 128)
                    dpos = jt - 4 * ic
                    # Causal: query columns below 128*dpos within this chunk
                    # see none of this key tile, so both QK matmuls narrow
                    # to the valid query range. Head 1's scores land at
                    # column 512 (adjacent to head 0's valid region) so one
                    # exp covers both halves with no dead zone.
                    ioff = 128 * dpos if dpos > 0 else 0
                    w = 512 - ioff
                    islw = slice(ic * 512 + ioff, (ic + 1) * 512)
                    nc.tensor.matmul(
                        sb[:, ioff:512],
                        lhsT=kT[pr][0:64, jsl],
                        rhs=qT[pr][0:64, islw],
                        start=True,
                        stop=True,
                        tile_position=(0, 0),
                    )
                    nc.tensor.matmul(
                        sb[:, 512 : 512 + w],
                        lhsT=kT[pr][64:128, jsl],
                        rhs=qT[pr][64:128, islw],
                        start=True,
                        stop=True,
                        tile_position=(64, 0),
                    )
                    nc.scalar.activation(
                        pTb[:, ioff : 512 + w],
                        sb[:, ioff : 512 + w],
                        EXP,
                        scale=0.125,
                    )
                    if dpos >= 0:
                        # mask[jj, c] = (c >= jj): all-ones beyond column
                        # 127, so only the staircase's first 128 columns
                        # need the multiply. Both heads' staircase blocks
                        # (at ioff and 512, one strided view) multiply in
                        # ONE DVE op against a region-broadcast mask.
                        mw = min(w, 128)
                        pv = two_region(pTb, ioff, 512 - ioff, mw)
                        nc.vector.tensor_mul(
                            pv, pv, mask_bcast(384, mw)
                        )
                    return ioff, w

                def pv_acc(pr, jt, pTb, prev_pTb, ioff, w, ps_out, pacc01):
                    first = jt == 0
                    last = jt == njt - 1
                    vt = v_sb[jt]
                    pT0 = pTb[:, ioff:512]
                    pT1 = pTb[:, 512 : 512 + w]
                    nc.tensor.matmul(
                        ps_out[0:64, ioff:512],
                        lhsT=vt[:, pr * 128 : pr * 128 + 64],
                        rhs=pT0,
                        start=first,
                        stop=False,
                        tile_position=(0, 0),
                        skip_group_check=True,
                    )
                    nc.tensor.matmul(
                        ps_out[64:128, ioff:512],
                        lhsT=vt[:, pr * 128 + 64 : pr * 128 + 128],
                        rhs=pT1,
                        start=first,
                        stop=last,
                        tile_position=(0, 64),
                        skip_group_check=True,
                    )
                    # p-sum accumulation: both heads share one [128,1024]
                    # pacc tile so a full-width j-tile accumulates with ONE
                    # 1024-wide DVE add. jt=0 is skipped (the jt=1 add reads
                    # both exp tiles); full-width adds ping-pong to a fresh
                    # buffer so they qualify for the non-in-place bf16 2x
                    # uop; diagonal-narrowed adds stay in place.
                    if first:
                        return pacc01
                    elif jt == 1:
                        if ioff == 0:
                            nc.vector.tensor_add(pacc01, prev_pTb, pTb)
                        else:
                            # Valid spans of both heads ([ioff:512] and
                            # [512+ioff:1024], stride 512) add in ONE
                            # strided op; ditto the dead-span copies.
                            # pacc/prev_pTb keep natural head-1 alignment
                            # (region stride 512); this jt's pTb packs
                            # head 1 at column 512 (stride 512-ioff).
                            av = two_region(pacc01, ioff, 512, 512 - ioff)
                            nc.vector.tensor_add(
                                av,
                                two_region(prev_pTb, ioff, 512, 512 - ioff),
                                two_region(pTb, ioff, 512 - ioff, 512 - ioff),
                            )
                            nc.vector.tensor_copy(
                                two_region(pacc01, 0, 512, ioff),
                                two_region(prev_pTb, 0, 512, ioff),
                            )
                    elif ioff == 0:
                        n01 = qpool.tile(
                            [128, 1024], bf16, name="pacc01", tag="pacc"
                        )
                        nc.vector.tensor_add(n01, pacc01, pTb)
                        return n01
                    else:
                        av = two_region(pacc01, ioff, 512, 512 - ioff)
                        nc.vector.tensor_add(
                            av,
                            av,
                            two_region(pTb, ioff, 512 - ioff, 512 - ioff),
                        )
                    return pacc01

                for g in duos:
                    duo = (2 * g, 2 * g + 1)
                    late = endgame and g == 1
                    ps_outs = {}
                    paccs = {}
                    for pr in duo:
                        ps_outs[pr] = ps_o.tile(
                            [128, 512], f32, name="ps_out", tag="pso"
                        )
                        paccs[pr] = qpool.tile(
                            [128, 1024], bf16, name="pacc01", tag="pacc"
                        )
                    prev = {}
                    for jt in range(njt):
                        # Both pairs' QK/exp emit before either pair's PV:
                        # the in-order PE queue then never holds a PV
                        # (blocked on its exp) ahead of the other pair's
                        # QK, so the two exps pipeline back-to-back.
                        step = {}
                        for pr in duo:
                            sb = ps_s.tile([128, 1024], f32, name="sb", tag="pss")
                            pTb = ppool.tile(
                                [128, 1024], bf16, name="pTb", tag="pT"
                            )
                            ioff, w = qk_exp(pr, jt, sb, pTb)
                            step[pr] = (pTb, ioff, w)
                        for pr in duo:
                            pTb, ioff, w = step[pr]
                            paccs[pr] = pv_acc(
                                pr,
                                jt,
                                pTb,
                                prev.get(pr),
                                ioff,
                                w,
                                ps_outs[pr],
                                paccs[pr],
                            )
                            prev[pr] = pTb
                    # Partition-reduce the accumulated p-sums into
                    # retired PV-output-ring slots: their ring successors
                    # (the next duo's ps_outs) are first needed at PV
                    # time, ~1.5us into that duo, which hides the
                    # denominator->reciprocal chain; the score ring stays
                    # a pure QK/exp cadence. Both pairs' den matmuls and
                    # unnormalized copies are emitted before either norm
                    # so the second pair's den is not queued behind the
                    # first pair's splat matmuls on the PE. The endgame
                    # duo puts head 1's sum at partition 64 (not 32) so
                    # the reciprocal rows line up with the (0,0)/(64,64)
                    # splat-matmul quadrants.
                    h1row = 64 if late else 32
                    dens = {}
                    for pr in duo:
                        den = ps_o.tile([128, 512], f32, name="den", tag="pso")
                        nc.tensor.matmul(
                            den[0:1, 0:512],
                            lhsT=ones_sb,
                            rhs=paccs[pr][:, 0:512],
                            start=True,
                            stop=True,
                            tile_position=(0, 0),
                            skip_group_check=True,
                        )
                        nc.tensor.matmul(
                            den[h1row : h1row + 1, 0:512],
                            lhsT=ones_sb,
                            rhs=paccs[pr][:, 512:1024],
                            start=True,
                            stop=True,
                            tile_position=(0, h1row),
                            skip_group_check=True,
                        )
                        # Unnormalized copy frees ps_out quickly;
                        # normalization happens in-place on att once the
                        # broadcast lands. The endgame duo's copies ride
                        # ScalarE (idle after the last exp) so DVE reaches
                        # the reciprocals sooner.
                        if late:
                            nc.scalar.copy(att[pr][:, isl], ps_outs[pr])
                        else:
                            nc.vector.tensor_copy(att[pr][:, isl], ps_outs[pr])
                        dens[pr] = den
                    for pr in duo:
                        norm_pair(pr, dens[pr])

            def phase_c(s, tail=False):
                """Output projection for token tiles 4s..4s+4.

                The tail call runs after the last attention chunk when the
                scalar engine is idle, so its psum evacuation moves there
                and the output DMAs split across both hardware queues.
                """
                for tt in range(4 * s, 4 * s + 4):
                    tsl = slice(tt * 128, (tt + 1) * 128)
                    for n in range(2):
                        ps = ps_a.tile([128, 512], f32, name="ps_c", tag="psA")
                        for fc in range(4):
                            nc.tensor.matmul(
                                ps,
                                lhsT=att[fc][:, tsl],
                                rhs=wo_sb[fc][:, n * 512 : (n + 1) * 512],
                                start=(fc == 0),
                                stop=(fc == 3),
                            )
                        ost = opool.tile([128, 512], f16, name="ost", tag="ost")
                        if tail:
                            nc.scalar.copy(ost, ps)
                            eng = nc.scalar if n else nc.sync
                        else:
                            nc.vector.tensor_copy(ost, ps)
                            eng = nc.sync
                        eng.dma_start(
                            out=out[tsl, n * 512 : (n + 1) * 512], in_=ost
                        )

            # Emission order interleaves so every B phase has PE filler
            # work available: B(s) overlaps A(s+1) (emitted just before)
            # and C(s-1) (emitted just after B(s) starts). B(3), the
            # longest attention chunk, gets C(2)+C(3). (Deferring ALL C
            # phases into B(3) was tried and regressed 35us: the extra
            # evacuation casts head-of-line block B3's mask multiplies on
            # the in-order DVE queue and stall the QK score ring.)
            # B0's first duo only needs q/k pairs 0-1 and v: emitting it
            # after half of A0 overlaps B0-duo0's exp-paced section with
            # A0's remaining q/k projection groups.
            phase_a(0, pairs_sel=(0, 1))
            phase_b(0, duo_sel=(0,))
            phase_a(0, pairs_sel=(2, 3), do_v=False)
            phase_b(0, duo_sel=(1,))
            phase_a(1)
            phase_b(1)
            phase_c(0)
            phase_a(2)
            phase_b(2)
            phase_c(1)
            phase_a(3)
            phase_b(3)
            phase_c(2)
            # Keep-warm dummies: these execute in the PE's only remaining
            # stall (the last duo's reciprocal->splat chain, which gates
            # C3 on att[2]/att[3]), so the final projection starts at the
            # fast p-state instead of ~1.6x slower.
            ps_w2 = ps_s.tile([128, 512], f32, name="ps_w2", tag="pss")
            for i in range(4):
                nc.tensor.matmul(
                    ps_w2,
                    lhsT=scratch[:, 0:128],
                    rhs=scratch[:],
                    start=(i == 0),
                    stop=(i == 3),
                )
            phase_c(3, tail=True)

    nc.compile()
    return nc


def _get_program():
    if "nc" not in _CACHE:
        _CACHE["nc"] = _build_program()
    return _CACHE["nc"]


def _make_mask():
    # msk[jj, z] = 1 if z >= jj + 384 else 0; diagonal-position-p mask
    # tile is msk[:, 384-128p : 384-128p+512].
    jj = np.arange(128)[:, None]
    z = np.arange(896)[None, :]
    return (z >= jj + 384).astype(ml_dtypes.bfloat16)


def _pmajor(a, np_dtype):
    """[N*128, F] -> partition-major [128, N, F] contiguous."""
    n = a.shape[0] // 128
    return np.ascontiguousarray(
        a.reshape(n, 128, a.shape[1]).transpose(1, 0, 2).astype(np_dtype)
    )


def _make_in_maps(x, w_qkv, w_out):
    mask = _make_mask()
    in_maps = []
    for core in range(NCORES):
        b, hg = core // 2, core % 2
        cs = slice(hg * 512, (hg + 1) * 512)
        f16 = np.float16
        xtb = np.ascontiguousarray(x[b].T).astype(f16)  # [C=1024, T=2048]
        # xt dram layout: [s, p, cc, c] = xtb[cc*128+p, s*512+c]
        xt4 = np.ascontiguousarray(
            xtb.reshape(CCH, 128, ICH, 512).transpose(2, 1, 0, 3)
        )
        in_maps.append(
            {
                "xt": xt4,
                "wq": _pmajor(w_qkv[:, hg * 512 : hg * 512 + 512], f16),
                "wk": _pmajor(
                    w_qkv[:, 1024 + hg * 512 : 1024 + hg * 512 + 512], f16
                ),
                "wv": _pmajor(
                    w_qkv[:, 2048 + hg * 512 : 2048 + hg * 512 + 512], f16
                ),
                "wo": _pmajor(w_out[cs, :], ml_dtypes.bfloat16),
                "msk": mask,
            }
        )
    return in_maps


def _run_device(in_maps, trace=False):
    from concourse.bass_utils import run_bass_kernel_spmd

    nc = _get_program()
    return run_bass_kernel_spmd(
        nc, in_maps, core_ids=list(range(NCORES)), trace=trace
    )


def kernel(x, w_qkv, w_out, b_out):
    x = np.asarray(x, dtype=np.float32)
    w_qkv = np.asarray(w_qkv, dtype=np.float32)
    w_out = np.asarray(w_out, dtype=np.float32)
    b_out = np.asarray(b_out, dtype=np.float32)

    res = _run_device(_make_in_maps(x, w_qkv, w_out)).results
    out = np.empty((B, T, C), dtype=np.float32)
    for b in range(B):
        out[b] = res[2 * b]["out"] + res[2 * b + 1]["out"] + b_out
    return out


# revision 23
# speedup vs baseline: 1.1709x; 1.1709x over previous
"""Multi-head causal self-attention on 8 Trainium2 NeuronCores.

Problem: x[4,2048,1024] @ w_qkv[1024,3072] -> 16-head causal attention
         -> @ w_out[1024,1024] + b_out.

Sharding (hardcoded): 8 cores = 4 batches x 2 head-groups of 8 heads.
Core c handles batch b = c//2 and heads hg*8..hg*8+8, hg = c%2.
Each core computes a partial output [2048,1024] (its 8 heads pushed
through its w_out row-slice); host sums the two head-group f16 partials
per batch and adds b_out.

Projections run in fp16; the attention pipeline (qT/kT/v/p/att/wo) is
bf16 so DVE tensor_tensor ops qualify for the 2x uop -- which only
triggers for non-in-place ops, hence the p-sum accumulation ping-pongs
between fresh buffers instead of updating in place. ~4e-3 relative
error vs the fp32 reference. (fp8 e4m3 QK via DoubleRow was evaluated
and rejected: ~3.3e-2 end-to-end error.)

Machine model (measured): every matmul streams 1 rhs row/cycle at
~2.4GHz; 64-deep pairs at tile positions (0,0)/(64,0) or (0,0)/(0,64)
stream BOTH rhs concurrently (2 rows/cycle aggregate), so QK/PV pairs
and the full-128-deep projection matmuls all run at full array rate.
The wall clock is essentially the PE queue makespan: streaming floor
~173us plus pipeline-drain exposure wherever a matmul waits on the
exp chain, plus startup/endgame. Hence the scheduling-oriented design:

  - Host pre-permutes every weight/x chunk so each input tensor is a
    partition-major [128, ...] DRAM block. Latency-critical tensors
    (wq, x^T chunk 0, wk, wv) load as piece-PAIR DMAs alternating
    across both hardware queues -- two 128-row pieces per ~565ns issue
    slot doubles the arrival rate while keeping fine-grained
    completion semaphores, so the first projection groups stream
    behind the DMAs (one bulk transfer per tensor stalled them ~5us
    on its single end-of-transfer semaphore). Far-from-critical
    tensors (x^T chunks 1-3, w_out, mask) load as one large DMA each,
    cutting ~35 DMA issue slots off the two queues.
  - Eight dummy matmuls on a memset scratch tile run while the first
    DMA pieces are in flight, plus four more in the endgame reciprocal
    stall, so the PE enters (and re-enters) its fast p-state before
    real work arrives (cold matmuls are ~1.6x slower).
  - Within each attention j-step, BOTH pairs' QK matmuls emit before
    either pair's PV: the in-order PE queue then never holds a PV
    (blocked on its exp) ahead of the other pair's QK, and the two
    exps pipeline back-to-back on ScalarE.
  - DVE (the second-busiest engine) op count is cut by fusing each
    diagonal tile's two per-head mask multiplies / p-sum adds into
    single strided two-region APs (~180ns fixed cost per op saved;
    head 1's exp output is PACKED at column 512, so its source regions
    use stride 512-ioff against the accumulator's natural stride 512).
  - opool bufs=8 so out-projection PE groups never wait on output-DMA
    ring slots.

Device algorithm per core (all "transposed orientation" so the only
transpose needed -- x^T -- is done for free on the host):
  qT/kT [512, 2048] and v (natural [2048, 512]) via fp16 matmuls.
  Attention runs per 512-query chunk with the two head-PAIRS of a duo
  interleaved j-tile by j-tile. Per pair, per key tile:
    scores^T for both heads go into one 2-bank PSUM tile with head 1's
    valid region packed adjacent to head 0's (no dead zone), both QK
    matmuls causally narrowed, streaming concurrently at PE tile
    positions (0,0)/(64,0). ONE exp per key tile on ScalarE covers
    both heads; 0/1-mask multiplies (DVE) only on diagonal-band tiles;
    out^T[d,i] += col-tiled PV matmuls at positions (0,0)/(0,64)
    (concurrent); p-sums accumulate in SBUF (DVE adds).
  Denominators: one M=1 ones-matmul per head partition-reduces the
  p-sum into a retired PV-output-ring PSUM slot; both pairs' den
  matmuls and unnormalized att copies emit before either pair's
  normalization so nothing queues behind the first pair's reciprocal
  chain. (GpSimd partition_all_reduce was measured at 8us per
  [128,1024] reduce -- far too slow to replace the ones-matmuls.)
  Per-pair normalization: reciprocal_approx_fast (DVE custom op), then
  a partition broadcast of the two reciprocal rows:
    - chunks 0-2 (hidden by later compute): strided 2-row DMA to a
      DRAM bounce, broadcast-read back to 128 partitions, in-place
      multiply into att^T.
    - chunk 3's last duo (nothing left to hide it): cast the two
      reciprocal rows to bf16 and splat them across partitions with
      two P=1 bf16 ones-matmuls into a retired PSUM slot (quadrants
      (0,0)/(64,64); head 1's denominator lands at partition 64 for
      this duo so the rows line up), and the in-place multiply reads
      PSUM directly. This cuts the final norm->out-projection critical
      path from ~7us (DMA round trip through DRAM) to ~2us; the
      unnormalized att copies for this duo ride ScalarE (idle after
      the last exp) so DVE reaches the reciprocals sooner. bf16
      reciprocals (vs f32) only touch 1/8th of the output at ~0.2%
      scale error -- noise against the 4e-3 overall error.
  partial = att^T.T @ w_out_slice -> f16 -> DMA to DRAM (the final
  projection call evacuates PSUM via ScalarE and splits its output
  DMAs across both queues, as ScalarE is idle after the last chunk).

PSUM budget (8 banks): score ring 2x[128,1024] = 4, shared
projection/QKV ring 2x[128,512] = 2, PV-output ring 2x[128,512] = 2.

Emission order A0h1 B0d0 A0h2 B0d1 A1 B1 C0 A2 B2 C1 A3 B3 C2 C3
keeps PE filler work (QKV projections / out-projections) available
inside every exp-paced attention chunk -- B(3), the longest, gets
C(2)+C(3) -- and splits A0/B0 so B0's first duo (which only needs q/k
pairs 0-1 and v) overlaps A0's remaining projection groups.
(Deferring ALL C phases into B(3) was tried and regressed 35us: the
extra evacuation casts head-of-line block B3's mask multiplies on the
in-order DVE queue and stall the QK score ring.)

Measured (this session's fast thermal regime): ~268-270us (very
stable at ~269.5) vs the ~283-285us session baseline; the part
intermittently throttles to a ~320us regime regardless of kernel
structure.
"""

import os
import sys

import ml_dtypes
import numpy as np

if "/opt/trn_rl_repo" not in sys.path:
    sys.path.insert(0, "/opt/trn_rl_repo")

B, T, C = 4, 2048, 1024
H, D = 16, 64
NCORES = 8
HPC = 8  # heads per core
PAIRS = 4  # head pairs per core
CCH = 8  # contraction chunks over C (1024/128)
ICH = 4  # i (query) chunks of 512
NJT = 16  # j (key) tiles of 128

_CACHE = {}


def _build_program():
    import concourse.mybir as mybir
    import concourse.tile as tile
    from concourse import bacc
    from concourse.bass import AP

    f32 = mybir.dt.float32
    bf16 = mybir.dt.bfloat16
    f16 = mybir.dt.float16
    EXP = mybir.ActivationFunctionType.Exp

    nc = bacc.Bacc(
        "TRN2", target_bir_lowering=False, debug=False, num_devices=NCORES
    )
    # Host pre-permutes so each DRAM tensor is partition-major: one
    # contiguous [128, ...] block per load (8KB descriptor lines).
    xt = nc.dram_tensor("xt", [ICH, 128, CCH, 512], f16, kind="ExternalInput").ap()
    wq = nc.dram_tensor("wq", [128, CCH, 512], f16, kind="ExternalInput").ap()
    wk = nc.dram_tensor("wk", [128, CCH, 512], f16, kind="ExternalInput").ap()
    wv = nc.dram_tensor("wv", [128, CCH, 512], f16, kind="ExternalInput").ap()
    wo = nc.dram_tensor("wo", [128, 4, C], bf16, kind="ExternalInput").ap()
    msk = nc.dram_tensor("msk", [128, 896], bf16, kind="ExternalInput").ap()
    out = nc.dram_tensor("out", [T, C], f16, kind="ExternalOutput").ap()

    with tile.TileContext(nc) as tc:
        with (
            tc.tile_pool(name="wpool", bufs=2) as wpool,
            tc.tile_pool(name="wvpool", bufs=1) as wvpool,
            tc.tile_pool(name="wopool", bufs=1) as wopool,
            tc.tile_pool(name="xpool", bufs=1) as xpool,
            tc.tile_pool(name="qkpool", bufs=8) as qkpool,
            tc.tile_pool(name="vpool", bufs=16) as vpool,
            tc.tile_pool(name="apool", bufs=4) as apool,
            tc.tile_pool(name="ppool", bufs=12) as ppool,
            tc.tile_pool(name="cpool", bufs=1) as cpool,
            tc.tile_pool(name="rpool", bufs=4) as rpool,
            tc.tile_pool(name="qpool", bufs=4) as qpool,
            tc.tile_pool(name="opool", bufs=8) as opool,
            tc.tile_pool(name="dpool", bufs=4, space="DRAM") as dpool,
            tc.tile_pool(name="ps_a", bufs=2, space="PSUM") as ps_a,
            tc.tile_pool(name="ps_s", bufs=2, space="PSUM") as ps_s,
            tc.tile_pool(name="ps_o", bufs=2, space="PSUM") as ps_o,
        ):
            # ---- constants ----
            ones_sb = cpool.tile([128, 1], bf16, name="ones_sb")
            nc.vector.memset(ones_sb, 1.0)
            # bf16 ones for the endgame reciprocal splat matmuls (bf16
            # keeps the splat single-pass; fp32 matmuls lower to a 2x
            # LOW/HIGH decomposition).
            ones_bf = cpool.tile([128, 64], bf16, name="ones_bf")
            nc.vector.memset(ones_bf, 1.0)
            # Scratch operand for PE warm-up matmuls (result discarded).
            scratch = cpool.tile([128, 512], bf16, name="scratch")
            nc.gpsimd.memset(scratch, 0.0)
            mask_sb = cpool.tile([128, 896], bf16, name="mask_sb")

            # PE warm-up: six dummy 512-row matmuls while the first input
            # pieces are still in flight, so the PE reaches its fast
            # p-state before the first real projection group.
            pass

            # Merged persistent input tiles.
            xt_all = xpool.tile([128, CCH, T], f16, name="xt_all", tag="xt")
            wq_all = wpool.tile([128, CCH, 512], f16, name="wq_all", tag="w")
            wk_all = wpool.tile([128, CCH, 512], f16, name="wk_all", tag="w")
            wv_all = wvpool.tile([128, CCH, 512], f16, name="wv_all", tag="wv")
            wo_all = wopool.tile([128, 4, C], bf16, name="wo_all", tag="wo")

            # wq + x^T chunk-0 pieces alternate across BOTH hardware
            # queues: matmul cc=k of the first group only needs piece k of
            # each, so the first A group streams behind the DMAs at the
            # two-queue rate.
            for cp in range(CCH // 2):
                ea, eb = (nc.sync, nc.scalar) if cp % 2 == 0 else (
                    nc.scalar,
                    nc.sync,
                )
                ea.dma_start(
                    out=wq_all[:, 2 * cp : 2 * cp + 2, :],
                    in_=wq[:, 2 * cp : 2 * cp + 2, :],
                )
                eb.dma_start(
                    out=xt_all[:, 2 * cp : 2 * cp + 2, 0:512],
                    in_=xt[0, :, 2 * cp : 2 * cp + 2, :],
                )
            # wk/wv stay piece-wise (alternating hardware queues) so the
            # k/v projection groups stream behind per-piece completion
            # semaphores instead of waiting on one bulk transfer. (GpSimd
            # SWDGE as a third issue queue was tried for wk and regressed
            # ~5us: ~1us/piece descriptor generation delivers too slowly.)
            for cp in range(CCH // 2):
                ea = nc.sync if cp % 2 == 0 else nc.scalar
                ea.dma_start(
                    out=wk_all[:, 2 * cp : 2 * cp + 2, :],
                    in_=wk[:, 2 * cp : 2 * cp + 2, :],
                )
            for cp in range(CCH // 2):
                ea = nc.scalar if cp % 2 == 0 else nc.sync
                ea.dma_start(
                    out=wv_all[:, 2 * cp : 2 * cp + 2, :],
                    in_=wv[:, 2 * cp : 2 * cp + 2, :],
                )
            nc.sync.dma_start(out=mask_sb, in_=msk)
            nc.scalar.dma_start(
                out=xt_all[:, :, 512:1024], in_=xt[1, :, :, :]
            )
            nc.sync.dma_start(
                out=xt_all[:, :, 1024:1536], in_=xt[2, :, :, :]
            )
            nc.scalar.dma_start(
                out=xt_all[:, :, 1536:2048], in_=xt[3, :, :, :]
            )
            nc.sync.dma_start(out=wo_all[:], in_=wo[:])

            w_sb = {}
            for cc in range(CCH):
                w_sb["wq", cc] = wq_all[:, cc, :]
                w_sb["wk", cc] = wk_all[:, cc, :]
                w_sb["wv", cc] = wv_all[:, cc, :]
            wo_sb = [wo_all[:, fc, :] for fc in range(4)]
            xt_sb = [xt_all[:, cc, :] for cc in range(CCH)]

            # ---- persistent activations ----
            qT = [
                qkpool.tile([128, T], bf16, name=f"qT_{p}", tag="qk")
                for p in range(PAIRS)
            ]
            kT = [
                qkpool.tile([128, T], bf16, name=f"kT_{p}", tag="qk")
                for p in range(PAIRS)
            ]
            v_sb = [
                vpool.tile([128, 512], bf16, name=f"v_{j}", tag="v")
                for j in range(NJT)
            ]
            att = [
                apool.tile([128, T], bf16, name=f"att_{p}", tag="att")
                for p in range(PAIRS)
            ]

            def phase_a(t4, pairs_sel=range(PAIRS), do_v=True):
                """QKV projections for token chunk t4 (512 tokens)."""
                tsl4 = slice(t4 * 512, (t4 + 1) * 512)
                xts = [xt_sb[cc][:, tsl4] for cc in range(CCH)]
                for wname, dst in (("wq", qT), ("wk", kT)):
                    for n in pairs_sel:
                        ps = ps_a.tile([128, 512], f32, name="ps_qk", tag="psA")
                        for cc in range(CCH):
                            nc.tensor.matmul(
                                ps,
                                lhsT=w_sb[wname, cc][:, n * 128 : (n + 1) * 128],
                                rhs=xts[cc][:],
                                start=(cc == 0),
                                stop=(cc == CCH - 1),
                            )
                        nc.vector.tensor_copy(
                            dst[n][:, t4 * 512 : (t4 + 1) * 512], ps
                        )
                if not do_v:
                    return
                for tt in range(4):
                    ps = ps_a.tile([128, 512], f32, name="ps_v", tag="psA")
                    for cc in range(CCH):
                        nc.tensor.matmul(
                            ps,
                            lhsT=xts[cc][:, tt * 128 : (tt + 1) * 128],
                            rhs=w_sb["wv", cc][:],
                            start=(cc == 0),
                            stop=(cc == CCH - 1),
                        )
                    nc.vector.tensor_copy(v_sb[t4 * 4 + tt], ps)

            def phase_b(ic, duo_sel=(0, 1)):
                """Attention for query chunk ic (512 queries).

                Pairs run as two interleaved duos (0,1) then (2,3): the
                j-tile loops of the duo alternate at emission so one pair's
                PV work hides the other pair's exp latency and the PE never
                idles long enough to drop out of its fast p-state.
                """
                isl = slice(ic * 512, (ic + 1) * 512)
                njt = 4 * ic + 4
                endgame = ic == 3
                duos = duo_sel

                def norm_pair(pr, den):
                    """1/denominators for pair pr -> broadcast + in-place mul."""
                    asl = att[pr][:, isl]
                    rec = rpool.tile([128, 1024], f32, name="rec", tag="rec")
                    if endgame and pr >= 2:
                        # Nothing left to hide a DRAM round trip behind:
                        # splat the two reciprocal rows (partitions 0/64)
                        # across partitions with two P=1 bf16 matmuls into
                        # a retired PSUM slot and multiply straight from
                        # PSUM.
                        nc.vector.reciprocal_approx_fast(
                            rec[0:65, 0:512], den[0:65, 0:512]
                        )
                        rec_bf = rpool.tile(
                            [128, 512], bf16, name="rec_bf", tag="recb"
                        )
                        nc.vector.tensor_copy(
                            rec_bf[0:1, :], rec[0:1, 0:512]
                        )
                        nc.vector.tensor_copy(
                            rec_bf[64:65, :], rec[64:65, 0:512]
                        )
                        rdb = ps_o.tile([128, 512], f32, name="rdb", tag="pso")
                        nc.tensor.matmul(
                            rdb[0:64, :],
                            lhsT=ones_bf[0:1, :],
                            rhs=rec_bf[0:1, :],
                            start=True,
                            stop=True,
                            tile_position=(0, 0),
                            skip_group_check=True,
                        )
                        nc.tensor.matmul(
                            rdb[64:128, :],
                            lhsT=ones_bf[64:65, :],
                            rhs=rec_bf[64:65, :],
                            start=True,
                            stop=True,
                            tile_position=(64, 64),
                            skip_group_check=True,
                        )
                        nc.vector.tensor_mul(asl, asl, rdb[:, 0:512])
                        return
                    eng = nc.sync
                    nc.vector.reciprocal_approx_fast(
                        rec[0:33, 0:512], den[0:33, 0:512]
                    )
                    dsc = dpool.tile([2, 512], f32, name="dsc", tag="dsc")
                    eng.dma_start(out=dsc, in_=rec[0:33:32, 0:512])
                    rdb = rpool.tile([128, 512], f32, name="rdb", tag="rdb")
                    eng.dma_start(
                        out=rdb[0:64, :],
                        in_=dsc[0:1, :].broadcast_to([64, 512]),
                    )
                    eng.dma_start(
                        out=rdb[64:128, :],
                        in_=dsc[1:2, :].broadcast_to([64, 512]),
                    )
                    nc.vector.tensor_mul(asl, asl, rdb)

                def two_region(t, off, rstride, width):
                    """[128, 2, width] view of t's two head spans at
                    offsets off and off+rstride."""
                    base = t[:, 0:1024]
                    return AP(
                        base.tensor,
                        base.offset + off,
                        [list(base.ap[0]), [rstride, 2], [1, width]],
                    )

                def mask_bcast(off, width):
                    base = mask_sb[:, off : off + width]
                    return AP(
                        base.tensor,
                        base.offset,
                        [list(base.ap[0]), [0, 2], [1, width]],
                    )

                def qk_exp(pr, jt, sb, pTb):
                    jsl = slice(jt * 128, (jt + 1) * 128)
                    dpos = jt - 4 * ic
                    # Causal: query columns below 128*dpos within this chunk
                    # see none of this key tile, so both QK matmuls narrow
                    # to the valid query range. Head 1's scores land at
                    # column 512 (adjacent to head 0's valid region) so one
                    # exp covers both halves with no dead zone.
                    ioff = 128 * dpos if dpos > 0 else 0
                    w = 512 - ioff
                    islw = slice(ic * 512 + ioff, (ic + 1) * 512)
                    nc.tensor.matmul(
                        sb[:, ioff:512],
                        lhsT=kT[pr][0:64, jsl],
                        rhs=qT[pr][0:64, islw],
                        start=True,
                        stop=True,
                        tile_position=(0, 0),
                    )
                    nc.tensor.matmul(
                        sb[:, 512 : 512 + w],
                        lhsT=kT[pr][64:128, jsl],
                        rhs=qT[pr][64:128, islw],
                        start=True,
                        stop=True,
                        tile_position=(64, 0),
                    )
                    nc.scalar.activation(
                        pTb[:, ioff : 512 + w],
                        sb[:, ioff : 512 + w],
                        EXP,
                        scale=0.125,
                    )
                    if dpos >= 0:
                        # mask[jj, c] = (c >= jj): all-ones beyond column
                        # 127, so only the staircase's first 128 columns
                        # need the multiply. Both heads' staircase blocks
                        # (at ioff and 512, one strided view) multiply in
                        # ONE DVE op against a region-broadcast mask.
                        mw = min(w, 128)
                        pv = two_region(pTb, ioff, 512 - ioff, mw)
                        nc.vector.tensor_mul(
                            pv, pv, mask_bcast(384, mw)
                        )
                    return ioff, w

                def pv_acc(pr, jt, pTb, prev_pTb, ioff, w, ps_out, pacc01):
                    first = jt == 0
                    last = jt == njt - 1
                    vt = v_sb[jt]
                    pT0 = pTb[:, ioff:512]
                    pT1 = pTb[:, 512 : 512 + w]
                    nc.tensor.matmul(
                        ps_out[0:64, ioff:512],
                        lhsT=vt[:, pr * 128 : pr * 128 + 64],
                        rhs=pT0,
                        start=first,
                        stop=False,
                        tile_position=(0, 0),
                        skip_group_check=True,
                    )
                    nc.tensor.matmul(
                        ps_out[64:128, ioff:512],
                        lhsT=vt[:, pr * 128 + 64 : pr * 128 + 128],
                        rhs=pT1,
                        start=first,
                        stop=last,
                        tile_position=(0, 64),
                        skip_group_check=True,
                    )
                    # p-sum accumulation: both heads share one [128,1024]
                    # pacc tile so a full-width j-tile accumulates with ONE
                    # 1024-wide DVE add. jt=0 is skipped (the jt=1 add reads
                    # both exp tiles); full-width adds ping-pong to a fresh
                    # buffer so they qualify for the non-in-place bf16 2x
                    # uop; diagonal-narrowed adds stay in place.
                    if first:
                        return pacc01
                    elif jt == 1:
                        if ioff == 0:
                            nc.vector.tensor_add(pacc01, prev_pTb, pTb)
                        else:
                            # Valid spans of both heads ([ioff:512] and
                            # [512+ioff:1024], stride 512) add in ONE
                            # strided op; ditto the dead-span copies.
                            # pacc/prev_pTb keep natural head-1 alignment
                            # (region stride 512); this jt's pTb packs
                            # head 1 at column 512 (stride 512-ioff).
                            av = two_region(pacc01, ioff, 512, 512 - ioff)
                            nc.vector.tensor_add(
                                av,
                                two_region(prev_pTb, ioff, 512, 512 - ioff),
                                two_region(pTb, ioff, 512 - ioff, 512 - ioff),
                            )
                            nc.vector.tensor_copy(
                                two_region(pacc01, 0, 512, ioff),
                                two_region(prev_pTb, 0, 512, ioff),
                            )
                    elif ioff == 0:
                        n01 = qpool.tile(
                            [128, 1024], bf16, name="pacc01", tag="pacc"
                        )
                        nc.vector.tensor_add(n01, pacc01, pTb)
                        return n01
                    else:
                        av = two_region(pacc01, ioff, 512, 512 - ioff)
                        nc.vector.tensor_add(
                            av,
                            av,
                            two_region(pTb, ioff, 512 - ioff, 512 - ioff),
                        )
                    return pacc01

                for g in duos:
                    duo = (2 * g, 2 * g + 1)
                    late = endgame and g == 1
                    ps_outs = {}
                    paccs = {}
                    for pr in duo:
                        ps_outs[pr] = ps_o.tile(
                            [128, 512], f32, name="ps_out", tag="pso"
                        )
                        paccs[pr] = qpool.tile(
                            [128, 1024], bf16, name="pacc01", tag="pacc"
                        )
                    prev = {}
                    for jt in range(njt):
                        # Both pairs' QK/exp emit before either pair's PV:
                        # the in-order PE queue then never holds a PV
                        # (blocked on its exp) ahead of the other pair's
                        # QK, so the two exps pipeline back-to-back.
                        step = {}
                        for pr in duo:
                            sb = ps_s.tile([128, 1024], f32, name="sb", tag="pss")
                            pTb = ppool.tile(
                                [128, 1024], bf16, name="pTb", tag="pT"
                            )
                            ioff, w = qk_exp(pr, jt, sb, pTb)
                            step[pr] = (pTb, ioff, w)
                        for pr in duo:
                            pTb, ioff, w = step[pr]
                            paccs[pr] = pv_acc(
                                pr,
                                jt,
                                pTb,
                                prev.get(pr),
                                ioff,
                                w,
                                ps_outs[pr],
                                paccs[pr],
                            )
                            prev[pr] = pTb
                    # Partition-reduce the accumulated p-sums into
                    # retired PV-output-ring slots: their ring successors
                    # (the next duo's ps_outs) are first needed at PV
                    # time, ~1.5us into that duo, which hides the
                    # denominator->reciprocal chain; the score ring stays
                    # a pure QK/exp cadence. Both pairs' den matmuls and
                    # unnormalized copies are emitted before either norm
                    # so the second pair's den is not queued behind the
                    # first pair's splat matmuls on the PE. The endgame
                    # duo puts head 1's sum at partition 64 (not 32) so
                    # the reciprocal rows line up with the (0,0)/(64,64)
                    # splat-matmul quadrants.
                    h1row = 64 if late else 32
                    dens = {}
                    for pr in duo:
                        den = ps_o.tile([128, 512], f32, name="den", tag="pso")
                        nc.tensor.matmul(
                            den[0:1, 0:512],
                            lhsT=ones_sb,
                            rhs=paccs[pr][:, 0:512],
                            start=True,
                            stop=True,
                            tile_position=(0, 0),
                            skip_group_check=True,
                        )
                        nc.tensor.matmul(
                            den[h1row : h1row + 1, 0:512],
                            lhsT=ones_sb,
                            rhs=paccs[pr][:, 512:1024],
                            start=True,
                            stop=True,
                            tile_position=(0, h1row),
                            skip_group_check=True,
                        )
                        # Unnormalized copy frees ps_out quickly;
                        # normalization happens in-place on att once the
                        # broadcast lands. The endgame duo's copies ride
                        # ScalarE (idle after the last exp) so DVE reaches
                        # the reciprocals sooner.
                        if late:
                            nc.scalar.copy(att[pr][:, isl], ps_outs[pr])
                        else:
                            nc.vector.tensor_copy(att[pr][:, isl], ps_outs[pr])
                        dens[pr] = den
                    for pr in duo:
                        norm_pair(pr, dens[pr])

            def phase_c(s, tail=False):
                """Output projection for token tiles 4s..4s+4.

                The tail call runs after the last attention chunk when the
                scalar engine is idle, so its psum evacuation moves there
                and the output DMAs split across both hardware queues.
                """
                for tt in range(4 * s, 4 * s + 4):
                    tsl = slice(tt * 128, (tt + 1) * 128)
                    for n in range(2):
                        ps = ps_a.tile([128, 512], f32, name="ps_c", tag="psA")
                        for fc in range(4):
                            nc.tensor.matmul(
                                ps,
                                lhsT=att[fc][:, tsl],
                                rhs=wo_sb[fc][:, n * 512 : (n + 1) * 512],
                                start=(fc == 0),
                                stop=(fc == 3),
                            )
                        ost = opool.tile([128, 512], f16, name="ost", tag="ost")
                        if tail:
                            nc.scalar.copy(ost, ps)
                            eng = nc.scalar if n else nc.sync
                        else:
                            nc.vector.tensor_copy(ost, ps)
                            eng = nc.sync
                        eng.dma_start(
                            out=out[tsl, n * 512 : (n + 1) * 512], in_=ost
                        )

            # Emission order interleaves so every B phase has PE filler
            # work available: B(s) overlaps A(s+1) (emitted just before)
            # and C(s-1) (emitted just after B(s) starts). B(3), the
            # longest attention chunk, gets C(2)+C(3). (Deferring ALL C
            # phases into B(3) was tried and regressed 35us: the extra
            # evacuation casts head-of-line block B3's mask multiplies on
            # the in-order DVE queue and stall the QK score ring.)
            # B0's first duo only needs q/k pairs 0-1 and v: emitting it
            # after half of A0 overlaps B0-duo0's exp-paced section with
            # A0's remaining q/k projection groups.
            phase_a(0, pairs_sel=(0, 1))
            phase_b(0, duo_sel=(0,))
            phase_a(0, pairs_sel=(2, 3), do_v=False)
            phase_b(0, duo_sel=(1,))
            phase_a(1)
            phase_b(1)
            phase_c(0)
            phase_a(2)
            phase_b(2)
            phase_c(1)
            phase_a(3)
            phase_b(3)
            phase_c(2)
            pass
            phase_c(3, tail=True)

    nc.compile()
    return nc


def _get_program():
    if "nc" not in _CACHE:
        _CACHE["nc"] = _build_program()
    return _CACHE["nc"]


def _make_mask():
    # msk[jj, z] = 1 if z >= jj + 384 else 0; diagonal-position-p mask
    # tile is msk[:, 384-128p : 384-128p+512].
    jj = np.arange(128)[:, None]
    z = np.arange(896)[None, :]
    return (z >= jj + 384).astype(ml_dtypes.bfloat16)


def _pmajor(a, np_dtype):
    """[N*128, F] -> partition-major [128, N, F] contiguous."""
    n = a.shape[0] // 128
    return np.ascontiguousarray(
        a.reshape(n, 128, a.shape[1]).transpose(1, 0, 2).astype(np_dtype)
    )


def _make_in_maps(x, w_qkv, w_out):
    mask = _make_mask()
    in_maps = []
    for core in range(NCORES):
        b, hg = core // 2, core % 2
        cs = slice(hg * 512, (hg + 1) * 512)
        f16 = np.float16
        xtb = np.ascontiguousarray(x[b].T).astype(f16)  # [C=1024, T=2048]
        # xt dram layout: [s, p, cc, c] = xtb[cc*128+p, s*512+c]
        xt4 = np.ascontiguousarray(
            xtb.reshape(CCH, 128, ICH, 512).transpose(2, 1, 0, 3)
        )
        in_maps.append(
            {
                "xt": xt4,
                "wq": _pmajor(w_qkv[:, hg * 512 : hg * 512 + 512], f16),
                "wk": _pmajor(
                    w_qkv[:, 1024 + hg * 512 : 1024 + hg * 512 + 512], f16
                ),
                "wv": _pmajor(
                    w_qkv[:, 2048 + hg * 512 : 2048 + hg * 512 + 512], f16
                ),
                "wo": _pmajor(w_out[cs, :], ml_dtypes.bfloat16),
                "msk": mask,
            }
        )
    return in_maps


def _run_device(in_maps, trace=False):
    from concourse.bass_utils import run_bass_kernel_spmd

    nc = _get_program()
    return run_bass_kernel_spmd(
        nc, in_maps, core_ids=list(range(NCORES)), trace=trace
    )


def kernel(x, w_qkv, w_out, b_out):
    x = np.asarray(x, dtype=np.float32)
    w_qkv = np.asarray(w_qkv, dtype=np.float32)
    w_out = np.asarray(w_out, dtype=np.float32)
    b_out = np.asarray(b_out, dtype=np.float32)

    res = _run_device(_make_in_maps(x, w_qkv, w_out)).results
    out = np.empty((B, T, C), dtype=np.float32)
    for b in range(B):
        out[b] = res[2 * b]["out"] + res[2 * b + 1]["out"] + b_out
    return out
